# revision 32
# baseline (speedup 1.0000x reference)
"""GQA attention kernel for Trainium2, 8 NeuronCores.

Sharding: TP-4 (kv-head pairs) x DP-2 (batch). Core c = b*4 + g handles
batch b, q-heads 8g..8g+7, kv-heads 2g..2g+1. Each core computes a partial
(D, S) output (its heads' contribution through wo); host sums the 4 partials
per batch.

All HBM traffic moves through large DMAs whose DRAM layouts are
host-swizzled so every transfer is one contiguous run per SBUF partition,
spread across the three DGE-capable queues so the initial loads overlap
(x/out on SP, wq/wo on GpSimd SWDGE, constants on Activation):
  - x arrives as xP[p, qc, kt, s], split into two kt-half DMAs per chunk
    so the first Q matmuls start as soon as the first half lands.
  - wq streams per chunk in two head-half pieces through a single buffer,
    each piece further split into two kt-half DMAs; the K-projection
    matmuls are emitted between the halves so the in-order PE queue
    covers the second piece's DMA.
  - constants are resident, loaded by TWO packed DMAs ordered by first
    use: [wk | cos | sin | tri | identity | ones] first, wv second.
  - wo streams once, in the single output-projection phase at the end,
    which consumes the attention output for the whole sequence (attnT_sb).
  - output staged per head-group in SBUF and written with two DMAs each.

RoPE uses de-interleaved q/k feature rows (evens on partitions 0-63, odds
on 64-127, via host-permuted wq/wk rows) so the pair rotation is two
half-height DVE muls with cross-partition operands — no SBUF-SBUF swap
DMAs. V is projected transposed (512-wide moving dim, like K) and
transposed back on the PE via identity matmuls — 72 matmuls/chunk
instead of 128. Scores are computed transposed (key, query) so PV needs
no transpose; the softmax denominator accumulates on the PE via a ones
matmul (broadcast form) and the normalization folds into the PSUM->SBUF
copy.

v2 changes vs baseline:
  - exact block-causal truncation: diagonal key tiles only compute the
    live query range (saves ~12% of attention-phase PE + ACT work), and
    the causal mask shrinks to one [128,128] triangular multiply on the
    strict-diagonal block.
  - reciprocal_approx_fast for the softmax denominator (5x faster than
    the iterative DVE reciprocal; removed the ~4.9us PE stall at every
    kv-head-pair boundary that also caused HAM re-throttling).
  - PSUM split into two explicit 2-buffer tag groups so attention
    score tiles never land on the PV/denominator accumulator banks.
  - PSUM->SBUF copies for the V path moved to the (otherwise idle)
    Scalar engine during projection phases.
  - startup DMA order: x/wq first, constants split so wk+rope tables
    arrive before first use and wv trails.
"""

import sys

if "/opt/trn_rl_repo" not in sys.path:
    sys.path.insert(0, "/opt/trn_rl_repo")

import math
import os

import ml_dtypes
import numpy as np

BF16 = ml_dtypes.bfloat16

B = 2
S = 2048
D = 4096
H = 32
KVH = 8
HD = 128
P = 128
TPG = 4                 # tensor-parallel groups (per batch)
LQH = H // TPG          # 8 local q heads
LKVH = KVH // TPG       # 2 local kv heads
QF = LQH * HD           # 1024 local q features
KF = LKVH * HD          # 256 local kv features
CHUNK = 512
NCHUNK = S // CHUNK     # 4
KT = D // P             # 32 contraction tiles for projections
SCALE = 1.0 / math.sqrt(HD)

# const layout: [cos | sin | tri | ident | ones | wv]; wk is its own
# tensor streamed on the GpSimd queue behind the first wq piece.
KW = KT * KF
NC1 = 2 * S + 3 * P           # first const DMA (rope tables, needed early)
NCONST = NC1 + KW             # + wv

_BUILT = None
LAST_EXEC_TIME_NS = None


def _build_program():
    import concourse.bass as bass  # noqa: F401
    import concourse.tile as tile
    from concourse import bacc, mybir

    nc = bacc.Bacc("TRN2", target_bir_lowering=False, debug=False,
                   num_devices=8)
    f32 = mybir.dt.float32
    b16 = mybir.dt.bfloat16

    xP = nc.dram_tensor("xP", [P, NCHUNK, KT, CHUNK], b16,
                        kind="ExternalInput").ap()
    # wqP[p, mh, kt, qf'] = wq_perm.T[kt*128+p, mh*512+qf']  (m-halves)
    wqP = nc.dram_tensor("wqP", [P, 2, KT, QF // 2], b16,
                         kind="ExternalInput").ap()
    wkP = nc.dram_tensor("wkP", [P, KT, KF], b16,
                         kind="ExternalInput").ap()
    woQ = nc.dram_tensor("woQ", [P, 8, LQH, CHUNK], b16,
                         kind="ExternalInput").ap()
    constP = nc.dram_tensor("constP", [P, NCONST], b16,
                            kind="ExternalInput").ap()
    # outP[p, qc, mt, s'] = partial_out[mt*128+p, qc*512+s']
    outP = nc.dram_tensor("outP", [P, NCHUNK, KT, CHUNK], b16,
                          kind="ExternalOutput").ap()

    Exp = mybir.ActivationFunctionType.Exp

    with tile.TileContext(nc) as tc:
        with (
            tc.tile_pool(name="consts", bufs=1) as consts,
            tc.tile_pool(name="persist", bufs=1) as persist,
            tc.tile_pool(name="qpool", bufs=1) as qpool,
            tc.tile_pool(name="stream", bufs=1) as stream,
            tc.tile_pool(name="work", bufs=1) as work,
            tc.tile_pool(name="ps", bufs=1, space="PSUM") as ps,
        ):
            # ---- constants: two ordered DMAs on the ACT queue (small rope
            # tables first, wv behind); wk streams on the GpSimd queue
            # after the first wq piece (emitted inside chunk 0)  ----
            # rope tables ride the ACT queue (small, needed ~45us); wk and
            # wv ride the sync FIFO behind chunk 0's x/wq (emitted in the
            # loop) so they don't steal startup bandwidth
            const_sb = consts.tile([P, NCONST], b16, name="const_sb")
            nc.scalar.dma_start(const_sb[:, :NC1], constP[:, :NC1])
            wk_t = consts.tile([P, KT, KF], b16, name="wk_t")

            cos_sb = const_sb[:, 0:S]
            sin_sb = const_sb[:, S:2 * S]
            tri_b = const_sb[:, 2 * S:2 * S + P]
            ident_b = const_sb[:, 2 * S + P:2 * S + 2 * P]
            ones_b = const_sb[:, 2 * S + 2 * P:2 * S + 3 * P]
            wv_sb = const_sb[:, NC1:].rearrange("p (k f) -> p k f", k=KT)
            wk_sb = wk_t

            # ---- persistent K^T (roped, de-interleaved rows) and V ----
            kT_sb = persist.tile([P, LKVH, S], b16, name="kT_sb")
            v_sb = persist.tile([P, S // P, KF], b16, name="v_sb")

            def psum_big(nm, tag):
                # two adjacent PSUM banks; "singles" are 512-wide windows
                return ps.tile([P, 2 * CHUNK], f32, tag=tag, bufs=2,
                               name=nm)

            def windows(big):
                return [big[:, :CHUNK], big[:, CHUNK:]]

            def rope(dst, src_psum, tsl, nm):
                """dst = rope(src_psum), de-interleaved feature layout.

                Partition p<64 holds even feature 2p ("a"), p>=64 holds odd
                feature 2(p-64)+1 ("b").  out_a = a*cos - b*sin,
                out_b = a*sin + b*cos.  cos_sb duplicates cos on both
                halves; sin_sb holds -sin on the top half, +sin on the
                bottom, so out[p] = src[p]*cos_sb[p] + src[p^64]*sin_sb[p].
                """
                tmp = work.tile([P, CHUNK], b16, tag="rtmp", bufs=1,
                                name=f"rt{nm}")
                nc.vector.tensor_mul(out=tmp[0:64, :], in0=src_psum[64:P, :],
                                     in1=sin_sb[0:64, tsl])
                nc.vector.tensor_mul(out=tmp[64:P, :], in0=src_psum[0:64, :],
                                     in1=sin_sb[64:P, tsl])
                nc.vector.tensor_mul(out=dst, in0=src_psum,
                                     in1=cos_sb[:, tsl])
                nc.vector.tensor_add(out=dst, in0=dst, in1=tmp)

            # attention output for the whole sequence; consumed by the
            # final wo phase so wo streams through SBUF exactly once.
            attnT_sb = qpool.tile([P, LQH, S], b16, name="attnT")

            for qc in range(NCHUNK):
                tsl = slice(qc * CHUNK, (qc + 1) * CHUNK)

                # ---- x + first wq piece for this chunk ----
                # On chunk 0 the first kt-slices of x and wq are interleaved
                # on the sync HWDGE queue (the earliest-starting one) so the
                # first Q matmul can issue ~12us in; the rest stream behind
                # on gpsimd/sync while the PE works.
                x_t = stream.tile([P, KT, CHUNK], b16, tag="xc", bufs=1,
                                  name=f"x{qc}")
                qT_sb = qpool.tile([P, LQH, CHUNK], b16, tag="qT", bufs=1,
                                   name=f"qT{qc}")
                wq_t0 = stream.tile([P, KT, QF // 2], b16, tag="wq", bufs=1,
                                    name=f"wq{qc}_0")
                if qc == 0:
                    # chunk 0 rides a single queue in exact need-order so
                    # no early bandwidth is stolen by later-needed pieces
                    # (the queue FIFO serializes; all 16 SDMA engines serve
                    # one queue at near-full HBM rate)
                    for h in range(4):
                        ksl = slice(h * 8, (h + 1) * 8)
                        nc.sync.dma_start(x_t[:, ksl], xP[:, qc, ksl])
                        nc.sync.dma_start(wq_t0[:, ksl], wqP[:, 0, ksl])
                    nc.sync.dma_start(wk_t, wkP)
                    # wv behind everything on the sync FIFO (needed ~70us)
                    nc.sync.dma_start(const_sb[:, NC1:], constP[:, NC1:])
                else:
                    nc.sync.dma_start(x_t[:, 0:KT // 2], xP[:, qc, 0:KT // 2])
                    nc.sync.dma_start(x_t[:, KT // 2:], xP[:, qc, KT // 2:])
                    nc.gpsimd.dma_start(wq_t0[:, 0:KT // 2],
                                        wqP[:, 0, 0:KT // 2])
                    nc.gpsimd.dma_start(wq_t0[:, KT // 2:],
                                        wqP[:, 0, KT // 2:])
                qb0 = [psum_big(f"qb0{qc}_{b}", "ps") for b in range(2)]
                qps0 = windows(qb0[0]) + windows(qb0[1])
                for kt in range(KT):
                    for mi in range(4):
                        nc.tensor.matmul(
                            qps0[mi],
                            wq_t0[:, kt, mi * P:(mi + 1) * P],
                            x_t[:, kt, :],
                            start=(kt == 0), stop=(kt == KT - 1))

                kps = psum_big(f"kb{qc}", "pso")
                kpw = windows(kps)
                for kt in range(KT):
                    for j in range(LKVH):
                        nc.tensor.matmul(
                            kpw[j],
                            wk_sb[:, kt, j * P:(j + 1) * P],
                            x_t[:, kt, :],
                            start=(kt == 0), stop=(kt == KT - 1))
                for mi in range(4):
                    rope(qT_sb[:, mi, :], qps0[mi], tsl, f"q{qc}_{mi}")

                wq_t1 = stream.tile([P, KT, QF // 2], b16, tag="wq", bufs=1,
                                    name=f"wq{qc}_1")
                nc.gpsimd.dma_start(wq_t1[:, 0:KT // 2], wqP[:, 1, 0:KT // 2])
                nc.gpsimd.dma_start(wq_t1[:, KT // 2:], wqP[:, 1, KT // 2:])
                qb1 = [psum_big(f"qb1{qc}_{b}", "ps") for b in range(2)]
                qps1 = windows(qb1[0]) + windows(qb1[1])
                for kt in range(KT):
                    for mi in range(4):
                        nc.tensor.matmul(
                            qps1[mi],
                            wq_t1[:, kt, mi * P:(mi + 1) * P],
                            x_t[:, kt, :],
                            start=(kt == 0), stop=(kt == KT - 1))
                for j in range(LKVH):
                    rope(kT_sb[:, j, tsl], kpw[j], tsl, f"k{qc}_{j}")

                # V projected transposed (vT, 512-wide moving like K), then
                # transposed back on the PE via identity matmuls — 72
                # matmuls/chunk instead of 128.
                vtps = psum_big(f"vtb{qc}", "pso")
                vtw = windows(vtps)
                for kt in range(KT):
                    for j in range(LKVH):
                        nc.tensor.matmul(
                            vtw[j],
                            wv_sb[:, kt, j * P:(j + 1) * P],
                            x_t[:, kt, :],
                            start=(kt == 0), stop=(kt == KT - 1))
                vT_sb = work.tile([P, LKVH, CHUNK], b16, tag="vT", bufs=1,
                                  name=f"vT{qc}")
                for j in range(LKVH):
                    nc.scalar.copy(out=vT_sb[:, j, :], in_=vtw[j])
                for mi in range(4):
                    rope(qT_sb[:, 4 + mi, :], qps1[mi], tsl, f"q{qc}_{4 + mi}")
                vpb = [psum_big(f"vpb{qc}_{b}", "pso") for b in range(2)]
                for st in range(4):
                    vp = windows(vpb[st // 2])[st % 2]
                    for j in range(LKVH):
                        nc.tensor.matmul(
                            vp[:, j * P:(j + 1) * P],
                            vT_sb[:, j, st * P:(st + 1) * P],
                            ident_b, start=True, stop=True)
                    nc.scalar.copy(
                        out=v_sb[:, qc * 4 + st, :], in_=vp[:, :KF])

                # ======== attention for this chunk ========
                # same-kv head PAIRS share a two-bank psum score tile, one
                # exp activation over both heads, and one V stationary.
                # Diagonal key tiles (kt == 4*qc + r, r in 0..3) only touch
                # the live query range [r*128, 512); the causal mask is one
                # [128,128] triangular multiply on the strict-diagonal
                # block.
                NT = 4 * qc + 4
                LOOK = 1
                flatp = [(pr, kt) for pr in range(4) for kt in range(NT)]
                exq = {}
                opvs = {}
                dpss = {}

                def emit_scores(pr, kt):
                    if (pr, kt) in exq:
                        return
                    kv = pr // 2
                    r = kt - 4 * qc
                    lo = max(r, 0) * P          # first live query col
                    sps = psum_big(f"sp{qc}_{pr}_{kt}", "ps")
                    for hi in range(2):
                        nc.tensor.matmul(
                            windows(sps)[hi][:, lo:],
                            kT_sb[:, kv, kt * P:(kt + 1) * P],
                            qT_sb[:, 2 * pr + hi, lo:],
                            start=True, stop=True)
                    ex = work.tile([P, 2 * CHUNK], b16, tag="exp", bufs=3,
                                   name=f"ex{qc}_{pr}_{kt}")
                    if lo == 0:
                        nc.scalar.activation(out=ex, in_=sps, func=Exp,
                                             scale=SCALE)
                    else:
                        for hi in range(2):
                            nc.scalar.activation(
                                out=ex[:, hi * CHUNK + lo:(hi + 1) * CHUNK],
                                in_=sps[:, hi * CHUNK + lo:(hi + 1) * CHUNK],
                                func=Exp, scale=SCALE)
                    if r >= 0:
                        for hi in range(2):
                            exd = ex[:, hi * CHUNK + r * P:
                                     hi * CHUNK + (r + 1) * P]
                            nc.vector.tensor_mul(out=exd, in0=exd, in1=tri_b)
                    exq[(pr, kt)] = (ex, lo)

                for pr, kt in flatp[:LOOK]:
                    emit_scores(pr, kt)
                for i, (pr, kt) in enumerate(flatp):
                    if i + LOOK < len(flatp):
                        emit_scores(*flatp[i + LOOK])
                    kv = pr // 2
                    if kt == 0:
                        # alternate slot roles per pr: the new PV accumulator
                        # lands on the banks the previous pr's reciprocals
                        # free first, not the ones its muls free last
                        if pr % 2 == 0:
                            opvs[pr] = psum_big(f"ov{qc}_{pr}", "pso")
                            dpss[pr] = psum_big(f"dp{qc}_{pr}", "pso")
                        else:
                            dpss[pr] = psum_big(f"dp{qc}_{pr}", "pso")
                            opvs[pr] = psum_big(f"ov{qc}_{pr}", "pso")
                    ex, lo = exq.pop((pr, kt))
                    for hi in range(2):
                        nc.tensor.matmul(
                            windows(opvs[pr])[hi][:, lo:],
                            v_sb[:, kt, kv * P:(kv + 1) * P],
                            ex[:, hi * CHUNK + lo:(hi + 1) * CHUNK],
                            start=(kt == 0), stop=(kt == NT - 1))
                    # denominator accumulates on PE: broadcast partition sum
                    for hi in range(2):
                        nc.tensor.matmul(
                            windows(dpss[pr])[hi][:, lo:], ones_b,
                            ex[:, hi * CHUNK + lo:(hi + 1) * CHUNK],
                            start=(kt == 0), stop=(kt == NT - 1))
                    if kt == NT - 1:
                        dps = dpss.pop(pr)
                        opv = opvs.pop(pr)
                        # both reciprocals first (frees the denominator
                        # banks for the next pr ~1.4us earlier), then the
                        # normalizing multiplies
                        drecs = []
                        for hi in range(2):
                            drec = work.tile([P, CHUNK], f32, tag="drec",
                                             bufs=2, name=f"dr{qc}_{pr}_{hi}")
                            nc.vector.reciprocal_approx_fast(
                                out=drec, in_=windows(dps)[hi])
                            drecs.append(drec)
                        for hi in range(2):
                            nc.vector.tensor_mul(
                                out=attnT_sb[:, 2 * pr + hi, tsl],
                                in0=windows(opv)[hi], in1=drecs[hi])

            # ======== output projection (all chunks; wo streamed once) ====
            for mtg in range(8):
                wo_t = stream.tile([P, LQH, CHUNK], b16, tag="wo",
                                   bufs=2, name=f"wo{mtg}")
                nc.gpsimd.dma_start(wo_t, woQ[:, mtg])
                osb = work.tile([P, 4, S], b16, tag="osb", bufs=1,
                                name=f"ou{mtg}")
                for qc in range(NCHUNK):
                    tsl = slice(qc * CHUNK, (qc + 1) * CHUNK)
                    opb = [psum_big(f"ob{qc}_{mtg}_{b}", t)
                           for b, t in ((0, "ps"), (1, "pso"))]
                    ops = windows(opb[0]) + windows(opb[1])
                    for kf in range(LQH):
                        for mi in range(4):
                            nc.tensor.matmul(
                                ops[mi],
                                wo_t[:, kf, mi * P:(mi + 1) * P],
                                attnT_sb[:, kf, tsl],
                                start=(kf == 0), stop=(kf == LQH - 1))
                    for b in range(2):
                        nc.scalar.copy(
                            out=osb[:, 2 * b:2 * b + 2, tsl],
                            in_=opb[b])
                    nc.sync.dma_start(
                        outP[:, qc, mtg * 4:(mtg + 1) * 4, :],
                        osb[:, :, tsl])

    nc.compile()
    return nc


def _deint(n_heads):
    """Row permutation de-interleaving head_dim pairs within each head."""
    idx = []
    for h in range(n_heads):
        base = h * HD
        idx.extend(base + 2 * i for i in range(HD // 2))
        idx.extend(base + 2 * i + 1 for i in range(HD // 2))
    return np.asarray(idx)


def _host_inputs(x, cos, sin, wq, wk, wv, wo):
    """Per-core input dicts. Core c = b*TPG + g."""
    cosT = np.ascontiguousarray(cos.T.astype(np.float32))   # (64, S)
    sinT = np.ascontiguousarray(sin.T.astype(np.float32))
    cosP = np.concatenate([cosT, cosT], axis=0).astype(BF16)  # (128, S)
    sinP = np.concatenate([-sinT, sinT], axis=0).astype(BF16)

    pp = np.arange(P)[:, None]
    ff = np.arange(P)[None, :]
    triP = (pp <= ff).astype(BF16)                          # (128, 128)

    qperm = _deint(LQH)
    kperm = _deint(LKVH)

    in_maps = []
    for c in range(8):
        b, g = divmod(c, TPG)
        qsl = slice(g * QF, (g + 1) * QF)
        ksl = slice(g * KF, (g + 1) * KF)

        # xP[p, qc, kt, s'] = x[b, qc*512+s', kt*128+p]
        xT = x[b].T.astype(BF16)                            # (D, S)
        xPc = np.ascontiguousarray(
            xT.reshape(KT, P, NCHUNK, CHUNK).transpose(1, 2, 0, 3))

        # wqP[p, mh, kt, qf'] = wq[qsl][qperm].T[kt*128+p, mh*512+qf']
        wqT = wq[qsl][qperm].T.astype(BF16)                 # (D, QF)
        wqPc = np.ascontiguousarray(
            wqT.reshape(KT, P, 2, QF // 2).transpose(1, 2, 0, 3))
        wkT = wk[ksl][kperm].T.astype(BF16)                 # (D, KF)
        wkPc = wkT.reshape(KT, P, KF).transpose(1, 0, 2)
        wvT = wv[ksl].T.astype(BF16)
        wvPc = wvT.reshape(KT, P, KF).transpose(1, 0, 2)
        constPc = np.concatenate([
            cosP, sinP,
            triP,
            np.eye(P, dtype=BF16),
            np.ones((P, P), dtype=BF16),
            wvPc.reshape(P, KT * KF),
        ], axis=1)

        # woQ[p, mtg, kf, dd] = wo[:, qsl].T[kf*128+p, mtg*512+dd]
        woT = wo[:, qsl].T.astype(BF16)                     # (QF, D)
        woQc = np.ascontiguousarray(
            woT.reshape(LQH, P, 8, CHUNK).transpose(1, 2, 0, 3))

        in_maps.append({
            "xP": xPc,
            "wqP": wqPc,
            "wkP": np.ascontiguousarray(wkPc),
            "woQ": woQc,
            "constP": np.ascontiguousarray(constPc),
        })
    return in_maps


def kernel(x, cos, sin, wq, wk, wv, wo):
    global _BUILT
    from concourse.bass_utils import run_bass_kernel_spmd

    x = np.asarray(x, np.float32)
    cos = np.asarray(cos, np.float32)
    sin = np.asarray(sin, np.float32)
    wq = np.asarray(wq, np.float32)
    wk = np.asarray(wk, np.float32)
    wv = np.asarray(wv, np.float32)
    wo = np.asarray(wo, np.float32)

    if _BUILT is None:
        _BUILT = _build_program()
    nc = _BUILT

    in_maps = _host_inputs(x, cos, sin, wq, wk, wv, wo)
    trace = os.environ.get("KERNEL_TRACE") == "1"
    try:
        res = run_bass_kernel_spmd(nc, in_maps, core_ids=list(range(8)),
                                   trace=trace)
    except Exception:
        if not trace:
            raise
        # profiling unavailable in this environment; run without it
        res = run_bass_kernel_spmd(nc, in_maps, core_ids=list(range(8)))
    global LAST_EXEC_TIME_NS
    LAST_EXEC_TIME_NS = getattr(res, "exec_time_ns", None)
    if trace and LAST_EXEC_TIME_NS is not None:
        print(f"HW exec time: {LAST_EXEC_TIME_NS} ns")

    out = np.zeros((B, S, D), np.float32)
    for c in range(8):
        b = c // TPG
        # outP[p, qc, mt, s'] -> partial (S, D)
        o = res.results[c]["outP"].astype(np.float32)
        o = o.transpose(2, 0, 1, 3).reshape(D, S)
        out[b] += o.T
    return out


# revision 33
# speedup vs baseline: 1.0032x; 1.0032x over previous
"""GQA attention kernel for Trainium2, 8 NeuronCores.

Sharding: TP-4 (kv-head pairs) x DP-2 (batch). Core c = b*4 + g handles
batch b, q-heads 8g..8g+7, kv-heads 2g..2g+1. Each core computes a partial
(D, S) output (its heads' contribution through wo); host sums the 4 partials
per batch.

All HBM traffic moves through large DMAs whose DRAM layouts are
host-swizzled so every transfer is one contiguous run per SBUF partition,
spread across the three DGE-capable queues so the initial loads overlap
(x/out on SP, wq/wo on GpSimd SWDGE, constants on Activation):
  - x arrives as xP[p, qc, kt, s], split into two kt-half DMAs per chunk
    so the first Q matmuls start as soon as the first half lands.
  - wq streams per chunk in two head-half pieces through a single buffer,
    each piece further split into two kt-half DMAs; the K-projection
    matmuls are emitted between the halves so the in-order PE queue
    covers the second piece's DMA.
  - constants are resident, loaded by TWO packed DMAs ordered by first
    use: [wk | cos | sin | tri | identity | ones] first, wv second.
  - wo streams once, in the single output-projection phase at the end,
    which consumes the attention output for the whole sequence (attnT_sb).
  - output staged per head-group in SBUF and written with two DMAs each.

RoPE uses de-interleaved q/k feature rows (evens on partitions 0-63, odds
on 64-127, via host-permuted wq/wk rows) so the pair rotation is two
half-height DVE muls with cross-partition operands — no SBUF-SBUF swap
DMAs. V is projected transposed (512-wide moving dim, like K) and
transposed back on the PE via identity matmuls — 72 matmuls/chunk
instead of 128. Scores are computed transposed (key, query) so PV needs
no transpose; the softmax denominator accumulates on the PE via a ones
matmul (broadcast form) and the normalization folds into the PSUM->SBUF
copy.

v2 changes vs baseline:
  - exact block-causal truncation: diagonal key tiles only compute the
    live query range (saves ~12% of attention-phase PE + ACT work), and
    the causal mask shrinks to one [128,128] triangular multiply on the
    strict-diagonal block.
  - reciprocal_approx_fast for the softmax denominator (5x faster than
    the iterative DVE reciprocal; removed the ~4.9us PE stall at every
    kv-head-pair boundary that also caused HAM re-throttling).
  - PSUM split into two explicit 2-buffer tag groups so attention
    score tiles never land on the PV/denominator accumulator banks.
  - PSUM->SBUF copies for the V path moved to the (otherwise idle)
    Scalar engine during projection phases.
  - startup DMA order: x/wq first, constants split so wk+rope tables
    arrive before first use and wv trails.
"""

import sys

if "/opt/trn_rl_repo" not in sys.path:
    sys.path.insert(0, "/opt/trn_rl_repo")

import math
import os

import ml_dtypes
import numpy as np

BF16 = ml_dtypes.bfloat16

B = 2
S = 2048
D = 4096
H = 32
KVH = 8
HD = 128
P = 128
TPG = 4                 # tensor-parallel groups (per batch)
LQH = H // TPG          # 8 local q heads
LKVH = KVH // TPG       # 2 local kv heads
QF = LQH * HD           # 1024 local q features
KF = LKVH * HD          # 256 local kv features
CHUNK = 512
NCHUNK = S // CHUNK     # 4
KT = D // P             # 32 contraction tiles for projections
SCALE = 1.0 / math.sqrt(HD)

# const layout: [cos | sin | tri | ident | ones | wv]; wk is its own
# tensor streamed on the GpSimd queue behind the first wq piece.
KW = KT * KF
NC1 = 2 * S + 3 * P           # first const DMA (rope tables, needed early)
NCONST = NC1 + KW             # + wv

_BUILT = None
LAST_EXEC_TIME_NS = None


def _build_program():
    import concourse.bass as bass  # noqa: F401
    import concourse.tile as tile
    from concourse import bacc, mybir

    nc = bacc.Bacc("TRN2", target_bir_lowering=False, debug=False,
                   num_devices=8)
    f32 = mybir.dt.float32
    b16 = mybir.dt.bfloat16

    xP = nc.dram_tensor("xP", [P, NCHUNK, KT, CHUNK], b16,
                        kind="ExternalInput").ap()
    # wqP[p, mh, kt, qf'] = wq_perm.T[kt*128+p, mh*512+qf']  (m-halves)
    wqP = nc.dram_tensor("wqP", [P, 2, KT, QF // 2], b16,
                         kind="ExternalInput").ap()
    wkP = nc.dram_tensor("wkP", [P, KT, KF], b16,
                         kind="ExternalInput").ap()
    woQ = nc.dram_tensor("woQ", [P, 8, LQH, CHUNK], b16,
                         kind="ExternalInput").ap()
    constP = nc.dram_tensor("constP", [P, NCONST], b16,
                            kind="ExternalInput").ap()
    # outP[p, qc, mt, s'] = partial_out[mt*128+p, qc*512+s']
    outP = nc.dram_tensor("outP", [P, NCHUNK, KT, CHUNK], b16,
                          kind="ExternalOutput").ap()

    Exp = mybir.ActivationFunctionType.Exp

    with tile.TileContext(nc) as tc:
        with (
            tc.tile_pool(name="consts", bufs=1) as consts,
            tc.tile_pool(name="persist", bufs=1) as persist,
            tc.tile_pool(name="qpool", bufs=1) as qpool,
            tc.tile_pool(name="stream", bufs=1) as stream,
            tc.tile_pool(name="work", bufs=1) as work,
            tc.tile_pool(name="ps", bufs=1, space="PSUM") as ps,
        ):
            # ---- constants: two ordered DMAs on the ACT queue (small rope
            # tables first, wv behind); wk streams on the GpSimd queue
            # after the first wq piece (emitted inside chunk 0)  ----
            # rope tables ride the ACT queue (small, needed ~45us); wk and
            # wv ride the sync FIFO behind chunk 0's x/wq (emitted in the
            # loop) so they don't steal startup bandwidth
            const_sb = consts.tile([P, NCONST], b16, name="const_sb")
            nc.scalar.dma_start(const_sb[:, :NC1], constP[:, :NC1])
            wk_t = consts.tile([P, KT, KF], b16, name="wk_t")

            cos_sb = const_sb[:, 0:S]
            sin_sb = const_sb[:, S:2 * S]
            tri_b = const_sb[:, 2 * S:2 * S + P]
            ident_b = const_sb[:, 2 * S + P:2 * S + 2 * P]
            ones_b = const_sb[:, 2 * S + 2 * P:2 * S + 3 * P]
            wv_sb = const_sb[:, NC1:].rearrange("p (k f) -> p k f", k=KT)
            wk_sb = wk_t

            # ---- persistent K^T (roped, de-interleaved rows) and V ----
            kT_sb = persist.tile([P, LKVH, S], b16, name="kT_sb")
            v_sb = persist.tile([P, S // P, KF], b16, name="v_sb")

            def psum_big(nm, tag):
                # two adjacent PSUM banks; "singles" are 512-wide windows
                return ps.tile([P, 2 * CHUNK], f32, tag=tag, bufs=2,
                               name=nm)

            def windows(big):
                return [big[:, :CHUNK], big[:, CHUNK:]]

            def rope(dst, src_psum, tsl, nm):
                """dst = rope(src_psum), de-interleaved feature layout.

                Partition p<64 holds even feature 2p ("a"), p>=64 holds odd
                feature 2(p-64)+1 ("b").  out_a = a*cos - b*sin,
                out_b = a*sin + b*cos.  cos_sb duplicates cos on both
                halves; sin_sb holds -sin on the top half, +sin on the
                bottom, so out[p] = src[p]*cos_sb[p] + src[p^64]*sin_sb[p].
                """
                tmp = work.tile([P, CHUNK], b16, tag="rtmp", bufs=1,
                                name=f"rt{nm}")
                nc.vector.tensor_mul(out=tmp[0:64, :], in0=src_psum[64:P, :],
                                     in1=sin_sb[0:64, tsl])
                nc.vector.tensor_mul(out=tmp[64:P, :], in0=src_psum[0:64, :],
                                     in1=sin_sb[64:P, tsl])
                nc.vector.tensor_mul(out=dst, in0=src_psum,
                                     in1=cos_sb[:, tsl])
                nc.vector.tensor_add(out=dst, in0=dst, in1=tmp)

            # attention output for the whole sequence; consumed by the
            # final wo phase so wo streams through SBUF exactly once.
            attnT_sb = qpool.tile([P, LQH, S], b16, name="attnT")

            for qc in range(NCHUNK):
                tsl = slice(qc * CHUNK, (qc + 1) * CHUNK)

                # ---- x + first wq piece for this chunk ----
                # On chunk 0 the first kt-slices of x and wq are interleaved
                # on the sync HWDGE queue (the earliest-starting one) so the
                # first Q matmul can issue ~12us in; the rest stream behind
                # on gpsimd/sync while the PE works.
                x_t = stream.tile([P, KT, CHUNK], b16, tag="xc", bufs=1,
                                  name=f"x{qc}")
                qT_sb = qpool.tile([P, LQH, CHUNK], b16, tag="qT", bufs=1,
                                   name=f"qT{qc}")
                wq_t0 = stream.tile([P, KT, QF // 2], b16, tag="wq", bufs=1,
                                    name=f"wq{qc}_0")
                if qc == 0:
                    # chunk 0 rides a single queue in exact need-order so
                    # no early bandwidth is stolen by later-needed pieces
                    # (the queue FIFO serializes; all 16 SDMA engines serve
                    # one queue at near-full HBM rate)
                    for h in range(4):
                        ksl = slice(h * 8, (h + 1) * 8)
                        nc.sync.dma_start(x_t[:, ksl], xP[:, qc, ksl])
                        nc.sync.dma_start(wq_t0[:, ksl], wqP[:, 0, ksl])
                    nc.sync.dma_start(wk_t, wkP)
                    # wv behind everything on the sync FIFO (needed ~70us)
                    nc.sync.dma_start(const_sb[:, NC1:], constP[:, NC1:])
                else:
                    nc.sync.dma_start(x_t[:, 0:KT // 2], xP[:, qc, 0:KT // 2])
                    nc.sync.dma_start(x_t[:, KT // 2:], xP[:, qc, KT // 2:])
                    nc.gpsimd.dma_start(wq_t0[:, 0:KT // 2],
                                        wqP[:, 0, 0:KT // 2])
                    nc.gpsimd.dma_start(wq_t0[:, KT // 2:],
                                        wqP[:, 0, KT // 2:])
                qb0 = [psum_big(f"qb0{qc}_{b}", "ps") for b in range(2)]
                qps0 = windows(qb0[0]) + windows(qb0[1])
                for kt in range(KT):
                    for mi in range(4):
                        nc.tensor.matmul(
                            qps0[mi],
                            wq_t0[:, kt, mi * P:(mi + 1) * P],
                            x_t[:, kt, :],
                            start=(kt == 0), stop=(kt == KT - 1))

                kps = psum_big(f"kb{qc}", "pso")
                kpw = windows(kps)
                for kt in range(KT):
                    for j in range(LKVH):
                        nc.tensor.matmul(
                            kpw[j],
                            wk_sb[:, kt, j * P:(j + 1) * P],
                            x_t[:, kt, :],
                            start=(kt == 0), stop=(kt == KT - 1))
                for mi in range(4):
                    rope(qT_sb[:, mi, :], qps0[mi], tsl, f"q{qc}_{mi}")

                wq_t1 = stream.tile([P, KT, QF // 2], b16, tag="wq", bufs=1,
                                    name=f"wq{qc}_1")
                nc.gpsimd.dma_start(wq_t1[:, 0:KT // 2], wqP[:, 1, 0:KT // 2])
                nc.gpsimd.dma_start(wq_t1[:, KT // 2:], wqP[:, 1, KT // 2:])
                qb1 = [psum_big(f"qb1{qc}_{b}", "ps") for b in range(2)]
                qps1 = windows(qb1[0]) + windows(qb1[1])
                for kt in range(KT):
                    for mi in range(4):
                        nc.tensor.matmul(
                            qps1[mi],
                            wq_t1[:, kt, mi * P:(mi + 1) * P],
                            x_t[:, kt, :],
                            start=(kt == 0), stop=(kt == KT - 1))
                for j in range(LKVH):
                    rope(kT_sb[:, j, tsl], kpw[j], tsl, f"k{qc}_{j}")

                # V projected transposed (vT, 512-wide moving like K), then
                # transposed back on the PE via identity matmuls — 72
                # matmuls/chunk instead of 128.
                vtps = psum_big(f"vtb{qc}", "pso")
                vtw = windows(vtps)
                for kt in range(KT):
                    for j in range(LKVH):
                        nc.tensor.matmul(
                            vtw[j],
                            wv_sb[:, kt, j * P:(j + 1) * P],
                            x_t[:, kt, :],
                            start=(kt == 0), stop=(kt == KT - 1))
                vT_sb = work.tile([P, LKVH, CHUNK], b16, tag="vT", bufs=1,
                                  name=f"vT{qc}")
                for j in range(LKVH):
                    nc.scalar.copy(out=vT_sb[:, j, :], in_=vtw[j])
                for mi in range(4):
                    rope(qT_sb[:, 4 + mi, :], qps1[mi], tsl, f"q{qc}_{4 + mi}")
                vpb = [psum_big(f"vpb{qc}_{b}", "pso") for b in range(2)]
                for st in range(4):
                    vp = windows(vpb[st // 2])[st % 2]
                    for j in range(LKVH):
                        nc.tensor.matmul(
                            vp[:, j * P:(j + 1) * P],
                            vT_sb[:, j, st * P:(st + 1) * P],
                            ident_b, start=True, stop=True)
                    nc.scalar.copy(
                        out=v_sb[:, qc * 4 + st, :], in_=vp[:, :KF])

                # ======== attention for this chunk ========
                # same-kv head PAIRS share a two-bank psum score tile, one
                # exp activation over both heads, and one V stationary.
                # Diagonal key tiles (kt == 4*qc + r, r in 0..3) only touch
                # the live query range [r*128, 512); the causal mask is one
                # [128,128] triangular multiply on the strict-diagonal
                # block.
                NT = 4 * qc + 4
                LOOK = 1
                flatp = [(pr, kt) for pr in range(4) for kt in range(NT)]
                exq = {}
                opvs = {}
                dpss = {}

                def emit_scores(pr, kt):
                    if (pr, kt) in exq:
                        return
                    kv = pr // 2
                    r = kt - 4 * qc
                    lo = max(r, 0) * P          # first live query col
                    sps = psum_big(f"sp{qc}_{pr}_{kt}", "ps")
                    for hi in range(2):
                        nc.tensor.matmul(
                            windows(sps)[hi][:, lo:],
                            kT_sb[:, kv, kt * P:(kt + 1) * P],
                            qT_sb[:, 2 * pr + hi, lo:],
                            start=True, stop=True)
                    ex = work.tile([P, 2 * CHUNK], b16, tag="exp", bufs=3,
                                   name=f"ex{qc}_{pr}_{kt}")
                    if lo == 0:
                        nc.scalar.activation(out=ex, in_=sps, func=Exp,
                                             scale=SCALE)
                    else:
                        for hi in range(2):
                            nc.scalar.activation(
                                out=ex[:, hi * CHUNK + lo:(hi + 1) * CHUNK],
                                in_=sps[:, hi * CHUNK + lo:(hi + 1) * CHUNK],
                                func=Exp, scale=SCALE)
                    if r >= 0:
                        for hi in range(2):
                            exd = ex[:, hi * CHUNK + r * P:
                                     hi * CHUNK + (r + 1) * P]
                            nc.vector.tensor_mul(out=exd, in0=exd, in1=tri_b)
                    exq[(pr, kt)] = (ex, lo)

                for pr, kt in flatp[:LOOK]:
                    emit_scores(pr, kt)
                for i, (pr, kt) in enumerate(flatp):
                    if i + LOOK < len(flatp):
                        emit_scores(*flatp[i + LOOK])
                    kv = pr // 2
                    if kt == 0:
                        opvs[pr] = psum_big(f"ov{qc}_{pr}", "pso")
                        dpss[pr] = psum_big(f"dp{qc}_{pr}", "pso")
                    ex, lo = exq.pop((pr, kt))
                    for hi in range(2):
                        nc.tensor.matmul(
                            windows(opvs[pr])[hi][:, lo:],
                            v_sb[:, kt, kv * P:(kv + 1) * P],
                            ex[:, hi * CHUNK + lo:(hi + 1) * CHUNK],
                            start=(kt == 0), stop=(kt == NT - 1))
                    # denominator accumulates on PE: broadcast partition sum
                    for hi in range(2):
                        nc.tensor.matmul(
                            windows(dpss[pr])[hi][:, lo:], ones_b,
                            ex[:, hi * CHUNK + lo:(hi + 1) * CHUNK],
                            start=(kt == 0), stop=(kt == NT - 1))
                    if kt == NT - 1:
                        dps = dpss.pop(pr)
                        opv = opvs.pop(pr)
                        # both reciprocals first (frees the denominator
                        # banks for the next pr ~1.4us earlier), then the
                        # normalizing multiplies
                        drecs = []
                        for hi in range(2):
                            drec = work.tile([P, CHUNK], f32, tag="drec",
                                             bufs=2, name=f"dr{qc}_{pr}_{hi}")
                            nc.vector.reciprocal_approx_fast(
                                out=drec, in_=windows(dps)[hi])
                            drecs.append(drec)
                        for hi in range(2):
                            nc.vector.tensor_mul(
                                out=attnT_sb[:, 2 * pr + hi, tsl],
                                in0=windows(opv)[hi], in1=drecs[hi])

            # ======== output projection (all chunks; wo streamed once) ====
            for mtg in range(8):
                wo_t = stream.tile([P, LQH, CHUNK], b16, tag="wo",
                                   bufs=2, name=f"wo{mtg}")
                nc.gpsimd.dma_start(wo_t, woQ[:, mtg])
                osb = work.tile([P, 4, S], b16, tag="osb", bufs=1,
                                name=f"ou{mtg}")
                for qc in range(NCHUNK):
                    tsl = slice(qc * CHUNK, (qc + 1) * CHUNK)
                    opb = [psum_big(f"ob{qc}_{mtg}_{b}", t)
                           for b, t in ((0, "ps"), (1, "pso"))]
                    ops = windows(opb[0]) + windows(opb[1])
                    for kf in range(LQH):
                        for mi in range(4):
                            nc.tensor.matmul(
                                ops[mi],
                                wo_t[:, kf, mi * P:(mi + 1) * P],
                                attnT_sb[:, kf, tsl],
                                start=(kf == 0), stop=(kf == LQH - 1))
                    for b in range(2):
                        nc.scalar.copy(
                            out=osb[:, 2 * b:2 * b + 2, tsl],
                            in_=opb[b])
                    nc.sync.dma_start(
                        outP[:, qc, mtg * 4:(mtg + 1) * 4, :],
                        osb[:, :, tsl])

    nc.compile()
    return nc


def _deint(n_heads):
    """Row permutation de-interleaving head_dim pairs within each head."""
    idx = []
    for h in range(n_heads):
        base = h * HD
        idx.extend(base + 2 * i for i in range(HD // 2))
        idx.extend(base + 2 * i + 1 for i in range(HD // 2))
    return np.asarray(idx)


def _host_inputs(x, cos, sin, wq, wk, wv, wo):
    """Per-core input dicts. Core c = b*TPG + g."""
    cosT = np.ascontiguousarray(cos.T.astype(np.float32))   # (64, S)
    sinT = np.ascontiguousarray(sin.T.astype(np.float32))
    cosP = np.concatenate([cosT, cosT], axis=0).astype(BF16)  # (128, S)
    sinP = np.concatenate([-sinT, sinT], axis=0).astype(BF16)

    pp = np.arange(P)[:, None]
    ff = np.arange(P)[None, :]
    triP = (pp <= ff).astype(BF16)                          # (128, 128)

    qperm = _deint(LQH)
    kperm = _deint(LKVH)

    in_maps = []
    for c in range(8):
        b, g = divmod(c, TPG)
        qsl = slice(g * QF, (g + 1) * QF)
        ksl = slice(g * KF, (g + 1) * KF)

        # xP[p, qc, kt, s'] = x[b, qc*512+s', kt*128+p]
        xT = x[b].T.astype(BF16)                            # (D, S)
        xPc = np.ascontiguousarray(
            xT.reshape(KT, P, NCHUNK, CHUNK).transpose(1, 2, 0, 3))

        # wqP[p, mh, kt, qf'] = wq[qsl][qperm].T[kt*128+p, mh*512+qf']
        wqT = wq[qsl][qperm].T.astype(BF16)                 # (D, QF)
        wqPc = np.ascontiguousarray(
            wqT.reshape(KT, P, 2, QF // 2).transpose(1, 2, 0, 3))
        wkT = wk[ksl][kperm].T.astype(BF16)                 # (D, KF)
        wkPc = wkT.reshape(KT, P, KF).transpose(1, 0, 2)
        wvT = wv[ksl].T.astype(BF16)
        wvPc = wvT.reshape(KT, P, KF).transpose(1, 0, 2)
        constPc = np.concatenate([
            cosP, sinP,
            triP,
            np.eye(P, dtype=BF16),
            np.ones((P, P), dtype=BF16),
            wvPc.reshape(P, KT * KF),
        ], axis=1)

        # woQ[p, mtg, kf, dd] = wo[:, qsl].T[kf*128+p, mtg*512+dd]
        woT = wo[:, qsl].T.astype(BF16)                     # (QF, D)
        woQc = np.ascontiguousarray(
            woT.reshape(LQH, P, 8, CHUNK).transpose(1, 2, 0, 3))

        in_maps.append({
            "xP": xPc,
            "wqP": wqPc,
            "wkP": np.ascontiguousarray(wkPc),
            "woQ": woQc,
            "constP": np.ascontiguousarray(constPc),
        })
    return in_maps


def kernel(x, cos, sin, wq, wk, wv, wo):
    global _BUILT
    from concourse.bass_utils import run_bass_kernel_spmd

    x = np.asarray(x, np.float32)
    cos = np.asarray(cos, np.float32)
    sin = np.asarray(sin, np.float32)
    wq = np.asarray(wq, np.float32)
    wk = np.asarray(wk, np.float32)
    wv = np.asarray(wv, np.float32)
    wo = np.asarray(wo, np.float32)

    if _BUILT is None:
        _BUILT = _build_program()
    nc = _BUILT

    in_maps = _host_inputs(x, cos, sin, wq, wk, wv, wo)
    trace = os.environ.get("KERNEL_TRACE") == "1"
    try:
        res = run_bass_kernel_spmd(nc, in_maps, core_ids=list(range(8)),
                                   trace=trace)
    except Exception:
        if not trace:
            raise
        # profiling unavailable in this environment; run without it
        res = run_bass_kernel_spmd(nc, in_maps, core_ids=list(range(8)))
    global LAST_EXEC_TIME_NS
    LAST_EXEC_TIME_NS = getattr(res, "exec_time_ns", None)
    if trace and LAST_EXEC_TIME_NS is not None:
        print(f"HW exec time: {LAST_EXEC_TIME_NS} ns")

    out = np.zeros((B, S, D), np.float32)
    for c in range(8):
        b = c // TPG
        # outP[p, qc, mt, s'] -> partial (S, D)
        o = res.results[c]["outP"].astype(np.float32)
        o = o.transpose(2, 0, 1, 3).reshape(D, S)
        out[b] += o.T
    return out


# revision 42
# speedup vs baseline: 1.0109x; 1.0076x over previous
"""GQA attention kernel for Trainium2, 8 NeuronCores.

Sharding: TP-4 (kv-head pairs) x DP-2 (batch). Core c = b*4 + g handles
batch b, q-heads 8g..8g+7, kv-heads 2g..2g+1. Each core computes a partial
(D, S) output (its heads' contribution through wo); host sums the 4 partials
per batch.

All HBM traffic moves through large DMAs whose DRAM layouts are
host-swizzled so every transfer is one contiguous run per SBUF partition,
spread across the three DGE-capable queues (x/out on SP, wq/wo on GpSimd
SWDGE, rope tables on Activation):
  - chunk 0's x and wq pieces ride the sync queue as a single FIFO in
    exact need-order (8-kt slices, interleaved), with wk and wv queued
    behind them, so no early bandwidth is stolen by later-needed data;
    later chunks use two kt-half DMAs per tensor, prefetched under the
    previous chunk's attention phase.
  - wq streams per chunk in two head-half pieces through a single buffer;
    the K-projection matmuls are emitted between the halves so the
    in-order PE queue covers the second piece's DMA.
  - wo streams once, in the single output-projection phase at the end,
    which consumes the attention output for the whole sequence (attnT_sb).
  - output staged per head-group in SBUF, written with one DMA per chunk
    as soon as that chunk's columns are copied out.

RoPE uses de-interleaved q/k feature rows (evens on partitions 0-63, odds
on 64-127, via host-permuted wq/wk rows) so the pair rotation is two
half-height DVE muls with cross-partition operands — no SBUF-SBUF swap
DMAs. V is projected transposed (512-wide moving dim, like K) and
transposed back on the PE via identity matmuls — 72 matmuls/chunk
instead of 128. Scores are computed transposed (key, query) so PV needs
no transpose; the softmax denominator accumulates on the PE via a ones
matmul (broadcast form) and the normalization folds into the PSUM->SBUF
copy.

Changes vs the original baseline (961us -> ~794us measured):
  - exact block-causal truncation: diagonal key tiles only compute the
    live query range (saves ~12% of attention-phase PE + ACT work), and
    the causal mask shrinks to one [128,128] triangular multiply on the
    strict-diagonal block.
  - reciprocal_approx_fast for the softmax denominator (5x faster than
    the iterative DVE reciprocal; removed the ~4.9us PE stall at every
    kv-head-pair boundary that also caused HAM re-throttling), with both
    reciprocals emitted before both normalizing muls so the denominator
    banks free early for the next head pair.
  - PSUM split into two explicit 2-buffer tag groups so attention
    score tiles never land on the PV/denominator accumulator banks.
  - PSUM->SBUF copies for the V path and output staging moved to the
    (otherwise idle) Scalar engine.
  - startup DMA need-ordering (first matmul at ~23us instead of ~52us)
    and per-chunk output DMAs to shrink the tail.
"""

import sys

if "/opt/trn_rl_repo" not in sys.path:
    sys.path.insert(0, "/opt/trn_rl_repo")

import math
import os

import ml_dtypes
import numpy as np

BF16 = ml_dtypes.bfloat16

B = 2
S = 2048
D = 4096
H = 32
KVH = 8
HD = 128
P = 128
TPG = 4                 # tensor-parallel groups (per batch)
LQH = H // TPG          # 8 local q heads
LKVH = KVH // TPG       # 2 local kv heads
QF = LQH * HD           # 1024 local q features
KF = LKVH * HD          # 256 local kv features
CHUNK = 512
NCHUNK = S // CHUNK     # 4
KT = D // P             # 32 contraction tiles for projections
SCALE = 1.0 / math.sqrt(HD)

# const layout: [cos | sin | tri | ident | ones | wv]; wk is its own
# tensor streamed on the GpSimd queue behind the first wq piece.
KW = KT * KF
NC1 = 2 * S + 3 * P           # first const DMA (rope tables, needed early)
NCONST = NC1 + KW             # + wv

_BUILT = None
LAST_EXEC_TIME_NS = None


def _build_program():
    import concourse.bass as bass  # noqa: F401
    import concourse.tile as tile
    from concourse import bacc, mybir

    nc = bacc.Bacc("TRN2", target_bir_lowering=False, debug=False,
                   num_devices=8)
    f32 = mybir.dt.float32
    b16 = mybir.dt.bfloat16

    xP = nc.dram_tensor("xP", [P, NCHUNK, KT, CHUNK], b16,
                        kind="ExternalInput").ap()
    # wqP[p, mh, kt, qf'] = wq_perm.T[kt*128+p, mh*512+qf']  (m-halves)
    wqP = nc.dram_tensor("wqP", [P, 2, KT, QF // 2], b16,
                         kind="ExternalInput").ap()
    wkP = nc.dram_tensor("wkP", [P, KT, KF], b16,
                         kind="ExternalInput").ap()
    woQ = nc.dram_tensor("woQ", [P, 8, LQH, CHUNK], b16,
                         kind="ExternalInput").ap()
    constP = nc.dram_tensor("constP", [P, NCONST], b16,
                            kind="ExternalInput").ap()
    # outP[p, qc, mt, s'] = partial_out[mt*128+p, qc*512+s']
    outP = nc.dram_tensor("outP", [P, NCHUNK, KT, CHUNK], b16,
                          kind="ExternalOutput").ap()

    Exp = mybir.ActivationFunctionType.Exp

    with tile.TileContext(nc) as tc:
        with (
            tc.tile_pool(name="consts", bufs=1) as consts,
            tc.tile_pool(name="persist", bufs=1) as persist,
            tc.tile_pool(name="qpool", bufs=1) as qpool,
            tc.tile_pool(name="stream", bufs=1) as stream,
            tc.tile_pool(name="work", bufs=1) as work,
            tc.tile_pool(name="ps", bufs=1, space="PSUM") as ps,
        ):
            # ---- constants: two ordered DMAs on the ACT queue (small rope
            # tables first, wv behind); wk streams on the GpSimd queue
            # after the first wq piece (emitted inside chunk 0)  ----
            # rope tables ride the ACT queue (small, needed ~45us); wk and
            # wv ride the sync FIFO behind chunk 0's x/wq (emitted in the
            # loop) so they don't steal startup bandwidth
            const_sb = consts.tile([P, NCONST], b16, name="const_sb")
            nc.scalar.dma_start(const_sb[:, :NC1], constP[:, :NC1])
            wk_t = consts.tile([P, KT, KF], b16, name="wk_t")

            cos_sb = const_sb[:, 0:S]
            sin_sb = const_sb[:, S:2 * S]
            tri_b = const_sb[:, 2 * S:2 * S + P]
            ident_b = const_sb[:, 2 * S + P:2 * S + 2 * P]
            ones_b = const_sb[:, 2 * S + 2 * P:2 * S + 3 * P]
            wv_sb = const_sb[:, NC1:].rearrange("p (k f) -> p k f", k=KT)
            wk_sb = wk_t

            # ---- persistent K^T (roped, de-interleaved rows) and V ----
            kT_sb = persist.tile([P, LKVH, S], b16, name="kT_sb")
            v_sb = persist.tile([P, S // P, KF], b16, name="v_sb")

            def psum_big(nm, tag):
                # two adjacent PSUM banks; "singles" are 512-wide windows
                return ps.tile([P, 2 * CHUNK], f32, tag=tag, bufs=2,
                               name=nm)

            def windows(big):
                return [big[:, :CHUNK], big[:, CHUNK:]]

            def rope(dst, src_psum, tsl, nm):
                """dst = rope(src_psum), de-interleaved feature layout.

                Partition p<64 holds even feature 2p ("a"), p>=64 holds odd
                feature 2(p-64)+1 ("b").  out_a = a*cos - b*sin,
                out_b = a*sin + b*cos.  cos_sb duplicates cos on both
                halves; sin_sb holds -sin on the top half, +sin on the
                bottom, so out[p] = src[p]*cos_sb[p] + src[p^64]*sin_sb[p].
                """
                tmp = work.tile([P, CHUNK], b16, tag="rtmp", bufs=1,
                                name=f"rt{nm}")
                nc.vector.tensor_mul(out=tmp[0:64, :], in0=src_psum[64:P, :],
                                     in1=sin_sb[0:64, tsl])
                nc.vector.tensor_mul(out=tmp[64:P, :], in0=src_psum[0:64, :],
                                     in1=sin_sb[64:P, tsl])
                nc.vector.tensor_mul(out=dst, in0=src_psum,
                                     in1=cos_sb[:, tsl])
                nc.vector.tensor_add(out=dst, in0=dst, in1=tmp)

            # attention output for the whole sequence; consumed by the
            # final wo phase so wo streams through SBUF exactly once.
            attnT_sb = qpool.tile([P, LQH, S], b16, name="attnT")

            for qc in range(NCHUNK):
                tsl = slice(qc * CHUNK, (qc + 1) * CHUNK)

                # ---- x + first wq piece for this chunk ----
                # On chunk 0 the first kt-slices of x and wq are interleaved
                # on the sync HWDGE queue (the earliest-starting one) so the
                # first Q matmul can issue ~12us in; the rest stream behind
                # on gpsimd/sync while the PE works.
                x_t = stream.tile([P, KT, CHUNK], b16, tag="xc", bufs=1,
                                  name=f"x{qc}")
                qT_sb = qpool.tile([P, LQH, CHUNK], b16, tag="qT", bufs=1,
                                   name=f"qT{qc}")
                wq_t0 = stream.tile([P, KT, QF // 2], b16, tag="wq", bufs=1,
                                    name=f"wq{qc}_0")
                if qc == 0:
                    # chunk 0 rides a single queue in exact need-order so
                    # no early bandwidth is stolen by later-needed pieces
                    # (the queue FIFO serializes; all 16 SDMA engines serve
                    # one queue at near-full HBM rate)
                    for h in range(4):
                        ksl = slice(h * 8, (h + 1) * 8)
                        nc.sync.dma_start(x_t[:, ksl], xP[:, qc, ksl])
                        nc.sync.dma_start(wq_t0[:, ksl], wqP[:, 0, ksl])
                    nc.sync.dma_start(wk_t, wkP)
                    # wv behind everything on the sync FIFO (needed ~70us)
                    nc.sync.dma_start(const_sb[:, NC1:], constP[:, NC1:])
                else:
                    for h in range(4):
                        xsl = slice(h * 8, (h + 1) * 8)
                        nc.sync.dma_start(x_t[:, xsl], xP[:, qc, xsl])
                    nc.gpsimd.dma_start(wq_t0[:, 0:KT // 2],
                                        wqP[:, 0, 0:KT // 2])
                    nc.gpsimd.dma_start(wq_t0[:, KT // 2:],
                                        wqP[:, 0, KT // 2:])
                qb0 = [psum_big(f"qb0{qc}_{b}", "ps") for b in range(2)]
                qps0 = windows(qb0[0]) + windows(qb0[1])
                for kt in range(KT):
                    for mi in range(4):
                        nc.tensor.matmul(
                            qps0[mi],
                            wq_t0[:, kt, mi * P:(mi + 1) * P],
                            x_t[:, kt, :],
                            start=(kt == 0), stop=(kt == KT - 1))

                kps = psum_big(f"kb{qc}", "pso")
                kpw = windows(kps)
                for kt in range(KT):
                    for j in range(LKVH):
                        nc.tensor.matmul(
                            kpw[j],
                            wk_sb[:, kt, j * P:(j + 1) * P],
                            x_t[:, kt, :],
                            start=(kt == 0), stop=(kt == KT - 1))
                for mi in range(4):
                    rope(qT_sb[:, mi, :], qps0[mi], tsl, f"q{qc}_{mi}")

                wq_t1 = stream.tile([P, KT, QF // 2], b16, tag="wq", bufs=1,
                                    name=f"wq{qc}_1")
                nc.gpsimd.dma_start(wq_t1[:, 0:KT // 2], wqP[:, 1, 0:KT // 2])
                nc.gpsimd.dma_start(wq_t1[:, KT // 2:], wqP[:, 1, KT // 2:])
                qb1 = [psum_big(f"qb1{qc}_{b}", "ps") for b in range(2)]
                qps1 = windows(qb1[0]) + windows(qb1[1])
                for kt in range(KT):
                    for mi in range(4):
                        nc.tensor.matmul(
                            qps1[mi],
                            wq_t1[:, kt, mi * P:(mi + 1) * P],
                            x_t[:, kt, :],
                            start=(kt == 0), stop=(kt == KT - 1))
                for j in range(LKVH):
                    rope(kT_sb[:, j, tsl], kpw[j], tsl, f"k{qc}_{j}")

                # V projected transposed (vT, 512-wide moving like K), then
                # transposed back on the PE via identity matmuls — 72
                # matmuls/chunk instead of 128.
                vtps = psum_big(f"vtb{qc}", "pso")
                vtw = windows(vtps)
                for kt in range(KT):
                    for j in range(LKVH):
                        nc.tensor.matmul(
                            vtw[j],
                            wv_sb[:, kt, j * P:(j + 1) * P],
                            x_t[:, kt, :],
                            start=(kt == 0), stop=(kt == KT - 1))
                vT_sb = work.tile([P, LKVH, CHUNK], b16, tag="vT", bufs=1,
                                  name=f"vT{qc}")
                for j in range(LKVH):
                    nc.scalar.copy(out=vT_sb[:, j, :], in_=vtw[j])
                for mi in range(4):
                    rope(qT_sb[:, 4 + mi, :], qps1[mi], tsl, f"q{qc}_{4 + mi}")
                vpb = [psum_big(f"vpb{qc}_{b}", "pso") for b in range(2)]
                for st in range(4):
                    vp = windows(vpb[st // 2])[st % 2]
                    for j in range(LKVH):
                        nc.tensor.matmul(
                            vp[:, j * P:(j + 1) * P],
                            vT_sb[:, j, st * P:(st + 1) * P],
                            ident_b, start=True, stop=True)
                    nc.scalar.copy(
                        out=v_sb[:, qc * 4 + st, :], in_=vp[:, :KF])

                # ======== attention for this chunk ========
                # same-kv head PAIRS share a two-bank psum score tile, one
                # exp activation over both heads, and one V stationary.
                # Diagonal key tiles (kt == 4*qc + r, r in 0..3) only touch
                # the live query range [r*128, 512); the causal mask is one
                # [128,128] triangular multiply on the strict-diagonal
                # block.
                NT = 4 * qc + 4
                LOOK = 1
                flatp = [(pr, kt) for pr in range(4) for kt in range(NT)]
                exq = {}
                opvs = {}
                dpss = {}

                def emit_scores(pr, kt):
                    if (pr, kt) in exq:
                        return
                    kv = pr // 2
                    r = kt - 4 * qc
                    lo = max(r, 0) * P          # first live query col
                    sps = psum_big(f"sp{qc}_{pr}_{kt}", "ps")
                    for hi in range(2):
                        nc.tensor.matmul(
                            windows(sps)[hi][:, lo:],
                            kT_sb[:, kv, kt * P:(kt + 1) * P],
                            qT_sb[:, 2 * pr + hi, lo:],
                            start=True, stop=True)
                    ex = work.tile([P, 2 * CHUNK], b16, tag="exp", bufs=4,
                                   name=f"ex{qc}_{pr}_{kt}")
                    if lo == 0:
                        nc.scalar.activation(out=ex, in_=sps, func=Exp,
                                             scale=SCALE)
                    else:
                        for hi in range(2):
                            nc.scalar.activation(
                                out=ex[:, hi * CHUNK + lo:(hi + 1) * CHUNK],
                                in_=sps[:, hi * CHUNK + lo:(hi + 1) * CHUNK],
                                func=Exp, scale=SCALE)
                    if r >= 0:
                        for hi in range(2):
                            exd = ex[:, hi * CHUNK + r * P:
                                     hi * CHUNK + (r + 1) * P]
                            nc.vector.tensor_mul(out=exd, in0=exd, in1=tri_b)
                    exq[(pr, kt)] = (ex, lo)

                for pr, kt in flatp[:LOOK]:
                    emit_scores(pr, kt)
                for i, (pr, kt) in enumerate(flatp):
                    if i + LOOK < len(flatp):
                        emit_scores(*flatp[i + LOOK])
                    kv = pr // 2
                    if kt == 0:
                        opvs[pr] = psum_big(f"ov{qc}_{pr}", "pso")
                        dpss[pr] = psum_big(f"dp{qc}_{pr}", "pso")
                    ex, lo = exq.pop((pr, kt))
                    for hi in range(2):
                        nc.tensor.matmul(
                            windows(opvs[pr])[hi][:, lo:],
                            v_sb[:, kt, kv * P:(kv + 1) * P],
                            ex[:, hi * CHUNK + lo:(hi + 1) * CHUNK],
                            start=(kt == 0), stop=(kt == NT - 1))
                    # denominator accumulates on PE: broadcast partition sum
                    for hi in range(2):
                        nc.tensor.matmul(
                            windows(dpss[pr])[hi][:, lo:], ones_b,
                            ex[:, hi * CHUNK + lo:(hi + 1) * CHUNK],
                            start=(kt == 0), stop=(kt == NT - 1))
                    if kt == NT - 1:
                        dps = dpss.pop(pr)
                        opv = opvs.pop(pr)
                        # both reciprocals first (frees the denominator
                        # banks for the next pr ~1.4us earlier), then the
                        # normalizing multiplies
                        drecs = []
                        for hi in range(2):
                            drec = work.tile([P, CHUNK], f32, tag="drec",
                                             bufs=2, name=f"dr{qc}_{pr}_{hi}")
                            nc.vector.reciprocal_approx_fast(
                                out=drec, in_=windows(dps)[hi])
                            drecs.append(drec)
                        for hi in range(2):
                            nc.vector.tensor_mul(
                                out=attnT_sb[:, 2 * pr + hi, tsl],
                                in0=windows(opv)[hi], in1=drecs[hi])

            # ======== output projection (all chunks; wo streamed once) ====
            wo_tiles = {}

            def fetch_wo(m):
                t = stream.tile([P, LQH, CHUNK], b16, tag="wo",
                                bufs=2, name=f"wo{m}")
                nc.scalar.dma_start(t, woQ[:, m])
                wo_tiles[m] = t

            fetch_wo(0)
            for mtg in range(8):
                wo_t = wo_tiles.pop(mtg)
                # osb is qc-major so every outP DMA reads one contiguous
                # 4KB run per partition (the [P,4,S] layout made the source
                # 4x1KB strided runs: 512 descriptors, ~70GB/s, and a 7us
                # tail on the very last transfer)
                osb = work.tile([P, NCHUNK, 4, CHUNK], b16, tag="osb",
                                bufs=1, name=f"ou{mtg}")
                for qc in range(NCHUNK):
                    tsl = slice(qc * CHUNK, (qc + 1) * CHUNK)
                    opb = [psum_big(f"ob{qc}_{mtg}_{b}", t)
                           for b, t in ((0, "ps"), (1, "pso"))]
                    ops = windows(opb[0]) + windows(opb[1])
                    for kf in range(LQH):
                        for mi in range(4):
                            nc.tensor.matmul(
                                ops[mi],
                                wo_t[:, kf, mi * P:(mi + 1) * P],
                                attnT_sb[:, kf, tsl],
                                start=(kf == 0), stop=(kf == LQH - 1))
                    # one staging copy on ACT, one on the (idle) DVE so the
                    # last copy before each DMA lands ~0.85us earlier
                    nc.scalar.copy(out=osb[:, qc, 0:2, :], in_=opb[0])
                    if qc == 0 and mtg + 1 < 8:
                        # prefetch the next weight group HERE in the ACT
                        # stream (~7us into this group) — emitted at the
                        # loop top it sits behind all four copies, each
                        # gated on its matmul group, and fires too late
                        fetch_wo(mtg + 1)
                    nc.vector.tensor_copy(out=osb[:, qc, 2:4, :],
                                          in_=opb[1])
                    nc.sync.dma_start(
                        outP[:, qc, mtg * 4:(mtg + 1) * 4, :],
                        osb[:, qc])

    nc.compile()
    return nc


def _deint(n_heads):
    """Row permutation de-interleaving head_dim pairs within each head."""
    idx = []
    for h in range(n_heads):
        base = h * HD
        idx.extend(base + 2 * i for i in range(HD // 2))
        idx.extend(base + 2 * i + 1 for i in range(HD // 2))
    return np.asarray(idx)


def _host_inputs(x, cos, sin, wq, wk, wv, wo):
    """Per-core input dicts. Core c = b*TPG + g."""
    cosT = np.ascontiguousarray(cos.T.astype(np.float32))   # (64, S)
    sinT = np.ascontiguousarray(sin.T.astype(np.float32))
    cosP = np.concatenate([cosT, cosT], axis=0).astype(BF16)  # (128, S)
    sinP = np.concatenate([-sinT, sinT], axis=0).astype(BF16)

    pp = np.arange(P)[:, None]
    ff = np.arange(P)[None, :]
    triP = (pp <= ff).astype(BF16)                          # (128, 128)

    qperm = _deint(LQH)
    kperm = _deint(LKVH)

    in_maps = []
    for c in range(8):
        b, g = divmod(c, TPG)
        qsl = slice(g * QF, (g + 1) * QF)
        ksl = slice(g * KF, (g + 1) * KF)

        # xP[p, qc, kt, s'] = x[b, qc*512+s', kt*128+p]
        xT = x[b].T.astype(BF16)                            # (D, S)
        xPc = np.ascontiguousarray(
            xT.reshape(KT, P, NCHUNK, CHUNK).transpose(1, 2, 0, 3))

        # wqP[p, mh, kt, qf'] = wq[qsl][qperm].T[kt*128+p, mh*512+qf']
        wqT = wq[qsl][qperm].T.astype(BF16)                 # (D, QF)
        wqPc = np.ascontiguousarray(
            wqT.reshape(KT, P, 2, QF // 2).transpose(1, 2, 0, 3))
        wkT = wk[ksl][kperm].T.astype(BF16)                 # (D, KF)
        wkPc = wkT.reshape(KT, P, KF).transpose(1, 0, 2)
        wvT = wv[ksl].T.astype(BF16)
        wvPc = wvT.reshape(KT, P, KF).transpose(1, 0, 2)
        constPc = np.concatenate([
            cosP, sinP,
            triP,
            np.eye(P, dtype=BF16),
            np.ones((P, P), dtype=BF16),
            wvPc.reshape(P, KT * KF),
        ], axis=1)

        # woQ[p, mtg, kf, dd] = wo[:, qsl].T[kf*128+p, mtg*512+dd]
        woT = wo[:, qsl].T.astype(BF16)                     # (QF, D)
        woQc = np.ascontiguousarray(
            woT.reshape(LQH, P, 8, CHUNK).transpose(1, 2, 0, 3))

        in_maps.append({
            "xP": xPc,
            "wqP": wqPc,
            "wkP": np.ascontiguousarray(wkPc),
            "woQ": woQc,
            "constP": np.ascontiguousarray(constPc),
        })
    return in_maps


def kernel(x, cos, sin, wq, wk, wv, wo):
    global _BUILT
    from concourse.bass_utils import run_bass_kernel_spmd

    x = np.asarray(x, np.float32)
    cos = np.asarray(cos, np.float32)
    sin = np.asarray(sin, np.float32)
    wq = np.asarray(wq, np.float32)
    wk = np.asarray(wk, np.float32)
    wv = np.asarray(wv, np.float32)
    wo = np.asarray(wo, np.float32)

    if _BUILT is None:
        _BUILT = _build_program()
    nc = _BUILT

    in_maps = _host_inputs(x, cos, sin, wq, wk, wv, wo)
    trace = os.environ.get("KERNEL_TRACE") == "1"
    try:
        res = run_bass_kernel_spmd(nc, in_maps, core_ids=list(range(8)),
                                   trace=trace)
    except Exception:
        if not trace:
            raise
        # profiling unavailable in this environment; run without it
        res = run_bass_kernel_spmd(nc, in_maps, core_ids=list(range(8)))
    global LAST_EXEC_TIME_NS
    LAST_EXEC_TIME_NS = getattr(res, "exec_time_ns", None)
    if trace and LAST_EXEC_TIME_NS is not None:
        print(f"HW exec time: {LAST_EXEC_TIME_NS} ns")

    out = np.zeros((B, S, D), np.float32)
    for c in range(8):
        b = c // TPG
        # outP[p, qc, mt, s'] -> partial (S, D)
        o = res.results[c]["outP"].astype(np.float32)
        o = o.transpose(2, 0, 1, 3).reshape(D, S)
        out[b] += o.T
    return out


# revision 43
# speedup vs baseline: 1.0124x; 1.0016x over previous
"""GQA attention kernel for Trainium2, 8 NeuronCores.

Sharding: TP-4 (kv-head pairs) x DP-2 (batch). Core c = b*4 + g handles
batch b, q-heads 8g..8g+7, kv-heads 2g..2g+1. Each core computes a partial
(D, S) output (its heads' contribution through wo); host sums the 4 partials
per batch.

All HBM traffic moves through large DMAs whose DRAM layouts are
host-swizzled so every transfer is one contiguous run per SBUF partition,
spread across the three DGE-capable queues (x/out on SP, wq/wo on GpSimd
SWDGE, rope tables on Activation):
  - chunk 0's x and wq pieces ride the sync queue as a single FIFO in
    exact need-order (8-kt slices, interleaved), with wk and wv queued
    behind them, so no early bandwidth is stolen by later-needed data;
    later chunks use two kt-half DMAs per tensor, prefetched under the
    previous chunk's attention phase.
  - wq streams per chunk in two head-half pieces through a single buffer;
    the K-projection matmuls are emitted between the halves so the
    in-order PE queue covers the second piece's DMA.
  - wo streams once, in the single output-projection phase at the end,
    which consumes the attention output for the whole sequence (attnT_sb).
  - output staged per head-group in SBUF, written with one DMA per chunk
    as soon as that chunk's columns are copied out.

RoPE uses de-interleaved q/k feature rows (evens on partitions 0-63, odds
on 64-127, via host-permuted wq/wk rows) so the pair rotation is two
half-height DVE muls with cross-partition operands — no SBUF-SBUF swap
DMAs. V is projected transposed (512-wide moving dim, like K) and
transposed back on the PE via identity matmuls — 72 matmuls/chunk
instead of 128. Scores are computed transposed (key, query) so PV needs
no transpose; the softmax denominator accumulates on the PE via a ones
matmul (broadcast form) and the normalization folds into the PSUM->SBUF
copy.

Changes vs the original baseline (961us -> ~794us measured):
  - exact block-causal truncation: diagonal key tiles only compute the
    live query range (saves ~12% of attention-phase PE + ACT work), and
    the causal mask shrinks to one [128,128] triangular multiply on the
    strict-diagonal block.
  - reciprocal_approx_fast for the softmax denominator (5x faster than
    the iterative DVE reciprocal; removed the ~4.9us PE stall at every
    kv-head-pair boundary that also caused HAM re-throttling), with both
    reciprocals emitted before both normalizing muls so the denominator
    banks free early for the next head pair.
  - PSUM split into two explicit 2-buffer tag groups so attention
    score tiles never land on the PV/denominator accumulator banks.
  - PSUM->SBUF copies for the V path and output staging moved to the
    (otherwise idle) Scalar engine.
  - startup DMA need-ordering (first matmul at ~23us instead of ~52us)
    and per-chunk output DMAs to shrink the tail.
"""

import sys

if "/opt/trn_rl_repo" not in sys.path:
    sys.path.insert(0, "/opt/trn_rl_repo")

import math
import os

import ml_dtypes
import numpy as np

BF16 = ml_dtypes.bfloat16

B = 2
S = 2048
D = 4096
H = 32
KVH = 8
HD = 128
P = 128
TPG = 4                 # tensor-parallel groups (per batch)
LQH = H // TPG          # 8 local q heads
LKVH = KVH // TPG       # 2 local kv heads
QF = LQH * HD           # 1024 local q features
KF = LKVH * HD          # 256 local kv features
CHUNK = 512
NCHUNK = S // CHUNK     # 4
KT = D // P             # 32 contraction tiles for projections
SCALE = 1.0 / math.sqrt(HD)

# const layout: [cos | sin | tri | ident | ones | wv]; wk is its own
# tensor streamed on the GpSimd queue behind the first wq piece.
KW = KT * KF
NC1 = 2 * S + 3 * P           # first const DMA (rope tables, needed early)
NCONST = NC1 + KW             # + wv

_BUILT = None
LAST_EXEC_TIME_NS = None


def _build_program():
    import concourse.bass as bass  # noqa: F401
    import concourse.tile as tile
    from concourse import bacc, mybir

    nc = bacc.Bacc("TRN2", target_bir_lowering=False, debug=False,
                   num_devices=8)
    f32 = mybir.dt.float32
    b16 = mybir.dt.bfloat16

    xP = nc.dram_tensor("xP", [P, NCHUNK, KT, CHUNK], b16,
                        kind="ExternalInput").ap()
    # wqP[p, mh, kt, qf'] = wq_perm.T[kt*128+p, mh*512+qf']  (m-halves)
    wqP = nc.dram_tensor("wqP", [P, 2, KT, QF // 2], b16,
                         kind="ExternalInput").ap()
    wkP = nc.dram_tensor("wkP", [P, KT, KF], b16,
                         kind="ExternalInput").ap()
    woQ = nc.dram_tensor("woQ", [P, 8, LQH, CHUNK], b16,
                         kind="ExternalInput").ap()
    constP = nc.dram_tensor("constP", [P, NCONST], b16,
                            kind="ExternalInput").ap()
    # outP[p, qc, mt, s'] = partial_out[mt*128+p, qc*512+s']
    outP = nc.dram_tensor("outP", [P, NCHUNK, KT, CHUNK], b16,
                          kind="ExternalOutput").ap()

    Exp = mybir.ActivationFunctionType.Exp

    with tile.TileContext(nc) as tc:
        with (
            tc.tile_pool(name="consts", bufs=1) as consts,
            tc.tile_pool(name="persist", bufs=1) as persist,
            tc.tile_pool(name="qpool", bufs=1) as qpool,
            tc.tile_pool(name="stream", bufs=1) as stream,
            tc.tile_pool(name="work", bufs=1) as work,
            tc.tile_pool(name="ps", bufs=1, space="PSUM") as ps,
        ):
            # ---- constants: two ordered DMAs on the ACT queue (small rope
            # tables first, wv behind); wk streams on the GpSimd queue
            # after the first wq piece (emitted inside chunk 0)  ----
            # rope tables ride the ACT queue (small, needed ~45us); wk and
            # wv ride the sync FIFO behind chunk 0's x/wq (emitted in the
            # loop) so they don't steal startup bandwidth
            const_sb = consts.tile([P, NCONST], b16, name="const_sb")
            nc.scalar.dma_start(const_sb[:, :NC1], constP[:, :NC1])
            wk_t = consts.tile([P, KT, KF], b16, name="wk_t")

            cos_sb = const_sb[:, 0:S]
            sin_sb = const_sb[:, S:2 * S]
            tri_b = const_sb[:, 2 * S:2 * S + P]
            ident_b = const_sb[:, 2 * S + P:2 * S + 2 * P]
            ones_b = const_sb[:, 2 * S + 2 * P:2 * S + 3 * P]
            wv_sb = const_sb[:, NC1:].rearrange("p (k f) -> p k f", k=KT)
            wk_sb = wk_t

            # ---- persistent K^T (roped, de-interleaved rows) and V ----
            kT_sb = persist.tile([P, LKVH, S], b16, name="kT_sb")
            v_sb = persist.tile([P, S // P, KF], b16, name="v_sb")

            def psum_big(nm, tag):
                # two adjacent PSUM banks; "singles" are 512-wide windows
                return ps.tile([P, 2 * CHUNK], f32, tag=tag, bufs=2,
                               name=nm)

            def windows(big):
                return [big[:, :CHUNK], big[:, CHUNK:]]

            def rope(dst, src_psum, tsl, nm):
                """dst = rope(src_psum), de-interleaved feature layout.

                Partition p<64 holds even feature 2p ("a"), p>=64 holds odd
                feature 2(p-64)+1 ("b").  out_a = a*cos - b*sin,
                out_b = a*sin + b*cos.  cos_sb duplicates cos on both
                halves; sin_sb holds -sin on the top half, +sin on the
                bottom, so out[p] = src[p]*cos_sb[p] + src[p^64]*sin_sb[p].
                """
                tmp = work.tile([P, CHUNK], b16, tag="rtmp", bufs=1,
                                name=f"rt{nm}")
                nc.vector.tensor_mul(out=tmp[0:64, :], in0=src_psum[64:P, :],
                                     in1=sin_sb[0:64, tsl])
                nc.vector.tensor_mul(out=tmp[64:P, :], in0=src_psum[0:64, :],
                                     in1=sin_sb[64:P, tsl])
                nc.vector.tensor_mul(out=dst, in0=src_psum,
                                     in1=cos_sb[:, tsl])
                nc.vector.tensor_add(out=dst, in0=dst, in1=tmp)

            # attention output for the whole sequence; consumed by the
            # final wo phase so wo streams through SBUF exactly once.
            attnT_sb = qpool.tile([P, LQH, S], b16, name="attnT")

            for qc in range(NCHUNK):
                tsl = slice(qc * CHUNK, (qc + 1) * CHUNK)

                # ---- x + first wq piece for this chunk ----
                # On chunk 0 the first kt-slices of x and wq are interleaved
                # on the sync HWDGE queue (the earliest-starting one) so the
                # first Q matmul can issue ~12us in; the rest stream behind
                # on gpsimd/sync while the PE works.
                x_t = stream.tile([P, KT, CHUNK], b16, tag="xc", bufs=1,
                                  name=f"x{qc}")
                qT_sb = qpool.tile([P, LQH, CHUNK], b16, tag="qT", bufs=1,
                                   name=f"qT{qc}")
                wq_t0 = stream.tile([P, KT, QF // 2], b16, tag="wq", bufs=1,
                                    name=f"wq{qc}_0")
                if qc == 0:
                    # chunk 0 rides a single queue in exact need-order so
                    # no early bandwidth is stolen by later-needed pieces
                    # (the queue FIFO serializes; all 16 SDMA engines serve
                    # one queue at near-full HBM rate)
                    for h in range(4):
                        ksl = slice(h * 8, (h + 1) * 8)
                        nc.sync.dma_start(x_t[:, ksl], xP[:, qc, ksl])
                        nc.sync.dma_start(wq_t0[:, ksl], wqP[:, 0, ksl])
                    nc.sync.dma_start(wk_t, wkP)
                    # wv behind everything on the sync FIFO (needed ~70us)
                    nc.sync.dma_start(const_sb[:, NC1:], constP[:, NC1:])
                else:
                    for h in range(4):
                        xsl = slice(h * 8, (h + 1) * 8)
                        nc.sync.dma_start(x_t[:, xsl], xP[:, qc, xsl])
                    nc.gpsimd.dma_start(wq_t0[:, 0:KT // 2],
                                        wqP[:, 0, 0:KT // 2])
                    nc.gpsimd.dma_start(wq_t0[:, KT // 2:],
                                        wqP[:, 0, KT // 2:])
                qb0 = [psum_big(f"qb0{qc}_{b}", "ps") for b in range(2)]
                qps0 = windows(qb0[0]) + windows(qb0[1])
                for kt in range(KT):
                    for mi in range(4):
                        nc.tensor.matmul(
                            qps0[mi],
                            wq_t0[:, kt, mi * P:(mi + 1) * P],
                            x_t[:, kt, :],
                            start=(kt == 0), stop=(kt == KT - 1))

                kps = psum_big(f"kb{qc}", "pso")
                kpw = windows(kps)
                for kt in range(KT):
                    for j in range(LKVH):
                        nc.tensor.matmul(
                            kpw[j],
                            wk_sb[:, kt, j * P:(j + 1) * P],
                            x_t[:, kt, :],
                            start=(kt == 0), stop=(kt == KT - 1))
                for mi in range(4):
                    rope(qT_sb[:, mi, :], qps0[mi], tsl, f"q{qc}_{mi}")

                wq_t1 = stream.tile([P, KT, QF // 2], b16, tag="wq", bufs=1,
                                    name=f"wq{qc}_1")
                nc.gpsimd.dma_start(wq_t1[:, 0:KT // 2], wqP[:, 1, 0:KT // 2])
                nc.gpsimd.dma_start(wq_t1[:, KT // 2:], wqP[:, 1, KT // 2:])
                qb1 = [psum_big(f"qb1{qc}_{b}", "ps") for b in range(2)]
                qps1 = windows(qb1[0]) + windows(qb1[1])
                for kt in range(KT):
                    for mi in range(4):
                        nc.tensor.matmul(
                            qps1[mi],
                            wq_t1[:, kt, mi * P:(mi + 1) * P],
                            x_t[:, kt, :],
                            start=(kt == 0), stop=(kt == KT - 1))
                for j in range(LKVH):
                    rope(kT_sb[:, j, tsl], kpw[j], tsl, f"k{qc}_{j}")

                # V projected transposed (vT, 512-wide moving like K), then
                # transposed back on the PE via identity matmuls — 72
                # matmuls/chunk instead of 128.
                vtps = psum_big(f"vtb{qc}", "pso")
                vtw = windows(vtps)
                for kt in range(KT):
                    for j in range(LKVH):
                        nc.tensor.matmul(
                            vtw[j],
                            wv_sb[:, kt, j * P:(j + 1) * P],
                            x_t[:, kt, :],
                            start=(kt == 0), stop=(kt == KT - 1))
                vT_sb = work.tile([P, LKVH, CHUNK], b16, tag="vT", bufs=1,
                                  name=f"vT{qc}")
                for j in range(LKVH):
                    nc.scalar.copy(out=vT_sb[:, j, :], in_=vtw[j])
                for mi in range(4):
                    rope(qT_sb[:, 4 + mi, :], qps1[mi], tsl, f"q{qc}_{4 + mi}")
                vpb = [psum_big(f"vpb{qc}_{b}", "pso") for b in range(2)]
                for st in range(4):
                    vp = windows(vpb[st // 2])[st % 2]
                    for j in range(LKVH):
                        nc.tensor.matmul(
                            vp[:, j * P:(j + 1) * P],
                            vT_sb[:, j, st * P:(st + 1) * P],
                            ident_b, start=True, stop=True)
                    nc.scalar.copy(
                        out=v_sb[:, qc * 4 + st, :], in_=vp[:, :KF])

                # ======== attention for this chunk ========
                # same-kv head PAIRS share a two-bank psum score tile, one
                # exp activation over both heads, and one V stationary.
                # Diagonal key tiles (kt == 4*qc + r, r in 0..3) only touch
                # the live query range [r*128, 512); the causal mask is one
                # [128,128] triangular multiply on the strict-diagonal
                # block.
                NT = 4 * qc + 4
                LOOK = 1
                flatp = [(pr, kt) for pr in range(4) for kt in range(NT)]
                exq = {}
                opvs = {}
                dpss = {}

                def emit_scores(pr, kt):
                    if (pr, kt) in exq:
                        return
                    kv = pr // 2
                    r = kt - 4 * qc
                    lo = max(r, 0) * P          # first live query col
                    sps = psum_big(f"sp{qc}_{pr}_{kt}", "ps")
                    for hi in range(2):
                        nc.tensor.matmul(
                            windows(sps)[hi][:, lo:],
                            kT_sb[:, kv, kt * P:(kt + 1) * P],
                            qT_sb[:, 2 * pr + hi, lo:],
                            start=True, stop=True)
                    ex = work.tile([P, 2 * CHUNK], b16, tag="exp", bufs=4,
                                   name=f"ex{qc}_{pr}_{kt}")
                    if lo == 0:
                        nc.scalar.activation(out=ex, in_=sps, func=Exp,
                                             scale=SCALE)
                    else:
                        for hi in range(2):
                            nc.scalar.activation(
                                out=ex[:, hi * CHUNK + lo:(hi + 1) * CHUNK],
                                in_=sps[:, hi * CHUNK + lo:(hi + 1) * CHUNK],
                                func=Exp, scale=SCALE)
                    if r >= 0:
                        for hi in range(2):
                            exd = ex[:, hi * CHUNK + r * P:
                                     hi * CHUNK + (r + 1) * P]
                            nc.vector.tensor_mul(out=exd, in0=exd, in1=tri_b)
                    exq[(pr, kt)] = (ex, lo)

                for pr, kt in flatp[:LOOK]:
                    emit_scores(pr, kt)
                for i, (pr, kt) in enumerate(flatp):
                    if i + LOOK < len(flatp):
                        emit_scores(*flatp[i + LOOK])
                    kv = pr // 2
                    if kt == 0:
                        opvs[pr] = psum_big(f"ov{qc}_{pr}", "pso")
                        dpss[pr] = psum_big(f"dp{qc}_{pr}", "pso")
                    ex, lo = exq.pop((pr, kt))
                    for hi in range(2):
                        nc.tensor.matmul(
                            windows(opvs[pr])[hi][:, lo:],
                            v_sb[:, kt, kv * P:(kv + 1) * P],
                            ex[:, hi * CHUNK + lo:(hi + 1) * CHUNK],
                            start=(kt == 0), stop=(kt == NT - 1))
                    # denominator accumulates on PE: broadcast partition sum
                    for hi in range(2):
                        nc.tensor.matmul(
                            windows(dpss[pr])[hi][:, lo:], ones_b,
                            ex[:, hi * CHUNK + lo:(hi + 1) * CHUNK],
                            start=(kt == 0), stop=(kt == NT - 1))
                    if kt == NT - 1:
                        dps = dpss.pop(pr)
                        opv = opvs.pop(pr)
                        # both reciprocals first (frees the denominator
                        # banks for the next pr ~1.4us earlier), then the
                        # normalizing multiplies
                        drecs = []
                        for hi in range(2):
                            drec = work.tile([P, CHUNK], f32, tag="drec",
                                             bufs=2, name=f"dr{qc}_{pr}_{hi}")
                            nc.vector.reciprocal_approx_fast(
                                out=drec, in_=windows(dps)[hi])
                            drecs.append(drec)
                        for hi in range(2):
                            nc.vector.tensor_mul(
                                out=attnT_sb[:, 2 * pr + hi, tsl],
                                in0=windows(opv)[hi], in1=drecs[hi])

            # ======== output projection (all chunks; wo streamed once) ====
            wo_tiles = {}

            def fetch_wo(m):
                t = stream.tile([P, LQH, CHUNK], b16, tag="wo",
                                bufs=2, name=f"wo{m}")
                nc.scalar.dma_start(t, woQ[:, m])
                wo_tiles[m] = t

            fetch_wo(0)
            for mtg in range(8):
                wo_t = wo_tiles.pop(mtg)
                # osb is qc-major so every outP DMA reads one contiguous
                # 4KB run per partition (the [P,4,S] layout made the source
                # 4x1KB strided runs: 512 descriptors, ~70GB/s, and a 7us
                # tail on the very last transfer)
                osb = work.tile([P, NCHUNK, 4, CHUNK], b16, tag="osb",
                                bufs=1, name=f"ou{mtg}")
                for qc in range(NCHUNK):
                    tsl = slice(qc * CHUNK, (qc + 1) * CHUNK)
                    opb = [psum_big(f"ob{qc}_{mtg}_{b}", t)
                           for b, t in ((0, "ps"), (1, "pso"))]
                    ops = windows(opb[0]) + windows(opb[1])
                    for kf in range(LQH):
                        for mi in range(4):
                            nc.tensor.matmul(
                                ops[mi],
                                wo_t[:, kf, mi * P:(mi + 1) * P],
                                attnT_sb[:, kf, tsl],
                                start=(kf == 0), stop=(kf == LQH - 1))
                    # one staging copy on ACT, one on the (idle) DVE so the
                    # last copy before each DMA lands ~0.85us earlier
                    nc.scalar.copy(out=osb[:, qc, 0:2, :], in_=opb[0])
                    if qc == 0 and mtg + 1 < 8:
                        # prefetch the next weight group HERE in the ACT
                        # stream (~7us into this group) — emitted at the
                        # loop top it sits behind all four copies, each
                        # gated on its matmul group, and fires too late
                        fetch_wo(mtg + 1)
                    if mtg == 7 and qc == NCHUNK - 1:
                        # final transfer of the program: split in two so the
                        # first half departs right after the ACT copy and
                        # only 256KB sits on the end-of-program critical
                        # path (copy -> DMA -> receipt -> barrier)
                        nc.sync.dma_start(
                            outP[:, qc, mtg * 4:mtg * 4 + 2, :],
                            osb[:, qc, 0:2])
                        nc.vector.tensor_copy(out=osb[:, qc, 2:4, :],
                                              in_=opb[1])
                        nc.sync.dma_start(
                            outP[:, qc, mtg * 4 + 2:mtg * 4 + 4, :],
                            osb[:, qc, 2:4])
                    else:
                        nc.vector.tensor_copy(out=osb[:, qc, 2:4, :],
                                              in_=opb[1])
                        nc.sync.dma_start(
                            outP[:, qc, mtg * 4:(mtg + 1) * 4, :],
                            osb[:, qc])

    nc.compile()
    return nc


def _deint(n_heads):
    """Row permutation de-interleaving head_dim pairs within each head."""
    idx = []
    for h in range(n_heads):
        base = h * HD
        idx.extend(base + 2 * i for i in range(HD // 2))
        idx.extend(base + 2 * i + 1 for i in range(HD // 2))
    return np.asarray(idx)


def _host_inputs(x, cos, sin, wq, wk, wv, wo):
    """Per-core input dicts. Core c = b*TPG + g."""
    cosT = np.ascontiguousarray(cos.T.astype(np.float32))   # (64, S)
    sinT = np.ascontiguousarray(sin.T.astype(np.float32))
    cosP = np.concatenate([cosT, cosT], axis=0).astype(BF16)  # (128, S)
    sinP = np.concatenate([-sinT, sinT], axis=0).astype(BF16)

    pp = np.arange(P)[:, None]
    ff = np.arange(P)[None, :]
    triP = (pp <= ff).astype(BF16)                          # (128, 128)

    qperm = _deint(LQH)
    kperm = _deint(LKVH)

    in_maps = []
    for c in range(8):
        b, g = divmod(c, TPG)
        qsl = slice(g * QF, (g + 1) * QF)
        ksl = slice(g * KF, (g + 1) * KF)

        # xP[p, qc, kt, s'] = x[b, qc*512+s', kt*128+p]
        xT = x[b].T.astype(BF16)                            # (D, S)
        xPc = np.ascontiguousarray(
            xT.reshape(KT, P, NCHUNK, CHUNK).transpose(1, 2, 0, 3))

        # wqP[p, mh, kt, qf'] = wq[qsl][qperm].T[kt*128+p, mh*512+qf']
        wqT = wq[qsl][qperm].T.astype(BF16)                 # (D, QF)
        wqPc = np.ascontiguousarray(
            wqT.reshape(KT, P, 2, QF // 2).transpose(1, 2, 0, 3))
        wkT = wk[ksl][kperm].T.astype(BF16)                 # (D, KF)
        wkPc = wkT.reshape(KT, P, KF).transpose(1, 0, 2)
        wvT = wv[ksl].T.astype(BF16)
        wvPc = wvT.reshape(KT, P, KF).transpose(1, 0, 2)
        constPc = np.concatenate([
            cosP, sinP,
            triP,
            np.eye(P, dtype=BF16),
            np.ones((P, P), dtype=BF16),
            wvPc.reshape(P, KT * KF),
        ], axis=1)

        # woQ[p, mtg, kf, dd] = wo[:, qsl].T[kf*128+p, mtg*512+dd]
        woT = wo[:, qsl].T.astype(BF16)                     # (QF, D)
        woQc = np.ascontiguousarray(
            woT.reshape(LQH, P, 8, CHUNK).transpose(1, 2, 0, 3))

        in_maps.append({
            "xP": xPc,
            "wqP": wqPc,
            "wkP": np.ascontiguousarray(wkPc),
            "woQ": woQc,
            "constP": np.ascontiguousarray(constPc),
        })
    return in_maps


def kernel(x, cos, sin, wq, wk, wv, wo):
    global _BUILT
    from concourse.bass_utils import run_bass_kernel_spmd

    x = np.asarray(x, np.float32)
    cos = np.asarray(cos, np.float32)
    sin = np.asarray(sin, np.float32)
    wq = np.asarray(wq, np.float32)
    wk = np.asarray(wk, np.float32)
    wv = np.asarray(wv, np.float32)
    wo = np.asarray(wo, np.float32)

    if _BUILT is None:
        _BUILT = _build_program()
    nc = _BUILT

    in_maps = _host_inputs(x, cos, sin, wq, wk, wv, wo)
    trace = os.environ.get("KERNEL_TRACE") == "1"
    try:
        res = run_bass_kernel_spmd(nc, in_maps, core_ids=list(range(8)),
                                   trace=trace)
    except Exception:
        if not trace:
            raise
        # profiling unavailable in this environment; run without it
        res = run_bass_kernel_spmd(nc, in_maps, core_ids=list(range(8)))
    global LAST_EXEC_TIME_NS
    LAST_EXEC_TIME_NS = getattr(res, "exec_time_ns", None)
    if trace and LAST_EXEC_TIME_NS is not None:
        print(f"HW exec time: {LAST_EXEC_TIME_NS} ns")

    out = np.zeros((B, S, D), np.float32)
    for c in range(8):
        b = c // TPG
        # outP[p, qc, mt, s'] -> partial (S, D)
        o = res.results[c]["outP"].astype(np.float32)
        o = o.transpose(2, 0, 1, 3).reshape(D, S)
        out[b] += o.T
    return out
